# revision 19
# baseline (speedup 1.0000x reference)
"""Cross-attention kernel for TRN2, 8-core SPMD.

Reference op (B=4, T=2048, S=512, D=1024, H=16, Hd=64):
    q = (x @ Wq + bq); k,v = context @ Wkv + bkv
    out = softmax(q k^T / sqrt(Hd) + mask) @ v @ Wp + bp

Sharding: pure data-parallel over (batch, T/2): core c owns batch c//2,
query rows (c%2)*1024..+1024.  Each core recomputes K/V for its batch
(2x duplicated KV-proj work, zero collectives).  Weights replicated.

Device design (per core, R=1024 query rows), activations flow in
"transposed" space (feature on partitions, rows on free):
  - K proj -> KT [D,S], Q proj -> QT [D,R] (fp16).
  - V proj -> V2 [S, D] fp16 in natural head order, wv pre-scaled 2^-10.
  - scores^T [S,R] per head: K=64 fp16 matmuls, two heads of a pair in
    PE row-groups 0/64 (concurrent), one ACT Exp per [128,1024] psum
    tile with the context mask folded in as a per-partition bias.
  - AV per (head-pair, rc-half): two M=64 accumulation chains (heads in
    col-groups 0/64, concurrent) into ONE psum bank -> a single
    [128,512] DVE eviction lands both heads aligned with OT; a second
    bank takes the softmax denominators via a shared ones(64) lhsT.
  - Normalization: denominators packed [128,2048] per 2-pair group,
    reciprocal on DVE (reciprocal_approx_fast, ~51 ULP) -- the ACT
    engine runs ONLY Exp (one table load, no LUT thrash), then one
    in-place DVE multiply per (hp, rc).
  - Y [R,D] = OT^T @ Wp + bp, fp32 out, DMA'd out on both queues.

Schedule: the ACT Exp stream (64 x 1.15us) is the pipeline clock for
the back half; PE filler (Q proj, V proj, AV) is spread across it via
emission order so the PE never idles long enough for HAM to re-throttle.
Input DMA is ordered ctx+wk (split across both queues) -> xT+wq -> wv,
with wp loaded late into the freed region, so K proj streams as early
as possible.

Numerics: fp16 operands everywhere, fp32 PSUM accumulation.  Max-abs
error vs the fp32 reference ~1e-3 of max|out|.
"""
import os
import sys
import types

import numpy as np

import concourse.tile as tile
from concourse import bacc, mybir
from concourse.bass_utils import run_bass_kernel_spmd

F32 = mybir.dt.float32
F32R = mybir.dt.float32r
F16 = mybir.dt.float16
AF = mybir.ActivationFunctionType

B, T, S, D = 4, 2048, 512, 1024
H, HD = 16, 64
NCORE = 8
R = B * T // NCORE          # 1024 query rows per core
KC = D // 128               # 8 contraction chunks
SC = S // 128               # 4 context chunks
NP = H // 2                 # 8 head pairs
NEG = -60.0                 # mask bias (exp(-60) ~ 0)

_CACHE = {}
last_results = None         # BassKernelResults of the most recent run


def _install_ntff_hook():
    """antenv.axon_hooks is absent in this image; recreate it from the
    boot helper so BASS_TRACE=1 profiling works. Best-effort."""
    try:
        import antenv.axon_hooks  # noqa: F401
        return
    except ImportError:
        pass
    try:
        from trn_agent_boot.trn_boot import _ntff_profile_via_ctypes
        hook = _ntff_profile_via_ctypes("/opt/axon/libaxon_pjrt.so")
        mod = types.ModuleType("antenv.axon_hooks")
        mod.get_axon_ntff_profile_hook = lambda: hook
        sys.modules["antenv.axon_hooks"] = mod
    except Exception:
        pass


_install_ntff_hook()


def _build():
    nc = bacc.Bacc("TRN2", target_bir_lowering=False, debug=False,
                   num_devices=NCORE)

    xT = nc.dram_tensor("xT", [D, R], F16, kind="ExternalInput").ap()
    ctxT = nc.dram_tensor("ctxT", [D, S], F16, kind="ExternalInput").ap()
    maskb = nc.dram_tensor("maskb", [128, SC], F32, kind="ExternalInput").ap()
    wq = nc.dram_tensor("wq", [D, D], F16, kind="ExternalInput").ap()
    bq = nc.dram_tensor("bq", [128, KC], F32, kind="ExternalInput").ap()
    wk = nc.dram_tensor("wk", [D, D], F16, kind="ExternalInput").ap()
    bk = nc.dram_tensor("bk", [128, KC], F32, kind="ExternalInput").ap()
    wv = nc.dram_tensor("wv", [D, D], F16, kind="ExternalInput").ap()
    wp = nc.dram_tensor("wp", [D, D], F16, kind="ExternalInput").ap()
    bp_r = nc.dram_tensor("bp_r", [128, D], F32, kind="ExternalInput").ap()
    ones64 = nc.dram_tensor("ones64", [128, 64], F16, kind="ExternalInput").ap()
    y = nc.dram_tensor("y", [R, D], F32, kind="ExternalOutput").ap()

    with tile.TileContext(nc) as tc:
        # Pools close LIFO (stack bottom -> top):
        #   const < kv < qt < ot < wpp < exp < sums < rcp < y
        #   < psAB < psQK < psAV < xT < wq < ctx < wv < wk
        # closes: wk (post kproj), wv+ctx (post vproj1), wq+xT (post
        # qproj67); then psAV+psQK close after attention and psD opens
        # for the output projection.
        p_const = tc.tile_pool(name="const", bufs=1)
        p_kv = tc.tile_pool(name="kv", bufs=1)
        p_qt = tc.tile_pool(name="qt", bufs=1)
        p_ot = tc.tile_pool(name="ot", bufs=1)
        p_wp = tc.tile_pool(name="wpp", bufs=1)
        p_exp = tc.tile_pool(name="exp", bufs=22)
        p_sums = tc.tile_pool(name="sums", bufs=1)
        p_rcp = tc.tile_pool(name="rcp", bufs=1)
        p_y = tc.tile_pool(name="y", bufs=2)
        p_psAB = tc.tile_pool(name="psAB", bufs=2, space="PSUM")
        p_psQK = tc.tile_pool(name="psQK", bufs=2, space="PSUM")
        p_psAV = tc.tile_pool(name="psAV", bufs=2, space="PSUM")
        p_xT = tc.tile_pool(name="xTp", bufs=1)
        p_wq = tc.tile_pool(name="wqp", bufs=1)
        p_ctx = tc.tile_pool(name="ctxp", bufs=1)
        p_wv = tc.tile_pool(name="wvp", bufs=1)
        p_wk = tc.tile_pool(name="wkp", bufs=1)
        constp = p_const.__enter__()
        kvp = p_kv.__enter__()
        qtp = p_qt.__enter__()
        otp = p_ot.__enter__()
        wpp = p_wp.__enter__()
        expp = p_exp.__enter__()
        sumsp = p_sums.__enter__()
        rcpp = p_rcp.__enter__()
        yp = p_y.__enter__()
        psAB = p_psAB.__enter__()
        psQK = p_psQK.__enter__()
        psAV = p_psAV.__enter__()
        xTp = p_xT.__enter__()
        wqp = p_wq.__enter__()
        ctxp = p_ctx.__enter__()
        wvp = p_wv.__enter__()
        wkp = p_wk.__enter__()

        # ---- PE warm-up on a memset tile: HAM warm before loads land;
        # a dummy Exp right away pulls the ACT table load off the
        # critical path. ----
        warm_sb = constp.tile([128, 256], F32R, tag="warm_sb")
        nc.vector.memset(warm_sb[:].bitcast(F32), 0.0)
        dummy_e = constp.tile([128, 16], F16, tag="dummy_e")
        nc.scalar.activation(dummy_e[:], warm_sb[:].bitcast(F32)[:, 0:16],
                             AF.Exp)
        warm_ps = psAB.tile([128, 512], F32, tag="psAB")
        for w in range(80):
            nc.tensor.matmul(warm_ps[:, 0:256], warm_sb[:, 0:128], warm_sb[:],
                             start=True, stop=True, skip_group_check=True)

        # ---- small constants FIRST: the KT/QT/AV evictions depend on
        # them, and behind megabytes of weight DMA they stall the DVE
        # queue for tens of us. ----
        mb_t = constp.tile([128, SC], F32, tag="mb")
        nc.sync.dma_start(mb_t[:], maskb[:])
        bq_t = constp.tile([128, KC], F32, tag="bq")
        nc.sync.dma_start(bq_t[:], bq[:])
        bk_t = constp.tile([128, KC], F32, tag="bk")
        nc.gpsimd.dma_start(bk_t[:], bk[:])
        ones_t = constp.tile([128, 64], F16, tag="ones")
        nc.gpsimd.dma_start(ones_t[:], ones64[:])

        # ---- phase A loads.  wk/wq are m-major on the host (tile m =
        # all k-chunks of output block m), so chain m only waits for its
        # own 256KB tile.  xT loads in rc-halves so qproj's rc0 chain
        # can start after 1MB.  Order: ctx -> wk0 -> xT(rc0) -> wq0 ->
        # xT(rc1) -> wk1/wq1 -> rest, split across both DMA queues. ----
        ctx_t = [ctxp.tile([128, S], F16, tag=f"ctx{k}", name=f"ctx{k}")
                 for k in range(KC)]
        wk_t = [wkp.tile([128, D], F16, tag=f"wk{m}", name=f"wk{m}")
                for m in range(KC)]
        xT_t = [xTp.tile([128, R], F16, tag=f"xT{k}", name=f"xTs{k}")
                for k in range(KC)]
        wq_t = [wqp.tile([128, D], F16, tag=f"wq{m}", name=f"wqs{m}")
                for m in range(KC)]
        for k in range(KC):
            eng = nc.sync if k % 2 == 0 else nc.gpsimd
            eng.dma_start(ctx_t[k][:], ctxT[k * 128:(k + 1) * 128, :])
        nc.sync.dma_start(wk_t[0][:], wk[0:128, :])
        for k in range(KC):
            eng = nc.gpsimd if k % 2 == 0 else nc.sync
            eng.dma_start(xT_t[k][:, 0:512], xT[k * 128:(k + 1) * 128, 0:512])
        nc.gpsimd.dma_start(wq_t[0][:], wq[0:128, :])
        for k in range(KC):
            eng = nc.gpsimd if k % 2 == 0 else nc.sync
            eng.dma_start(xT_t[k][:, 512:1024],
                          xT[k * 128:(k + 1) * 128, 512:1024])
        nc.sync.dma_start(wk_t[1][:], wk[128:256, :])
        nc.gpsimd.dma_start(wq_t[1][:], wq[128:256, :])
        # wv next: vproj0's deadline (av0 at exp-hp1-end) is EARLIER
        # than kproj/qproj chains 2-7
        wv_t = [wvp.tile([128, D], F16, tag=f"wv{k}", name=f"wv{k}")
                for k in range(KC)]
        for k in range(KC):
            eng = nc.sync if k % 2 == 0 else nc.gpsimd
            eng.dma_start(wv_t[k][:], wv[k * 128:(k + 1) * 128, :])
        for m in range(2, KC):
            eng = nc.sync if m % 2 == 0 else nc.gpsimd
            eng.dma_start(wk_t[m][:], wk[m * 128:(m + 1) * 128, :])
            eng2 = nc.gpsimd if m % 2 == 0 else nc.sync
            eng2.dma_start(wq_t[m][:], wq[m * 128:(m + 1) * 128, :])

        # ---- persistent attention operands (fp16) ----
        KT = [kvp.tile([128, S], F16, tag=f"KT{m}", name=f"KT{m}")
              for m in range(KC)]
        V2 = [kvp.tile([128, H * HD], F16, tag=f"V2{s}", name=f"V2{s}")
              for s in range(SC)]
        QT = [qtp.tile([128, R], F16, tag=f"QT{m}", name=f"QT{m}")
              for m in range(KC)]
        OT = [otp.tile([128, R], F16, tag=f"OT{m}", name=f"OT{m}")
              for m in range(KC)]
        wp_t = [wpp.tile([128, D], F16, tag=f"wp{k}", name=f"wps{k}")
                for k in range(KC)]

        # ---- emitters.  wk_t/wq_t tile m holds ALL k-chunks of output
        # block m: lhsT for the k-th matmul is tile[m][:, k*128:+128].
        def k_proj(ms):
            for m in ms:
                ps = psAB.tile([128, S], F32, tag="psAB")
                for k in range(KC):
                    nc.tensor.matmul(ps[:], wk_t[m][:, k * 128:(k + 1) * 128],
                                     ctx_t[k][:],
                                     start=(k == 0), stop=(k == KC - 1))
                nc.vector.tensor_scalar_add(KT[m][:], ps[:], bk_t[:, m:m + 1])

        def q_proj(ms):
            for m in ms:
                for rc in range(2):
                    ps = psAB.tile([128, 512], F32, tag="psAB")
                    for k in range(KC):
                        nc.tensor.matmul(
                            ps[:], wq_t[m][:, k * 128:(k + 1) * 128],
                            xT_t[k][:, rc * 512:(rc + 1) * 512],
                            start=(k == 0), stop=(k == KC - 1))
                    nc.vector.tensor_scalar_add(
                        QT[m][:, rc * 512:(rc + 1) * 512], ps[:],
                        bq_t[:, m:m + 1])

        def v_proj(n):
            for s in range(SC):
                ps = psAB.tile([128, 512], F32, tag="psAB")
                for k in range(KC):
                    nc.tensor.matmul(ps[:], ctx_t[k][:, s * 128:(s + 1) * 128],
                                     wv_t[k][:, n * 512:(n + 1) * 512],
                                     start=(k == 0), stop=(k == KC - 1))
                nc.vector.tensor_copy(V2[s][:, n * 512:(n + 1) * 512], ps[:])

        def attn_qk(hp):
            """scores^T + exp for one head pair; both heads run in PE
            row-groups 0/64 concurrently.  Returns ex[e][s] fp16 tiles."""
            ex = [[expp.tile([128, R], F16, tag="exp", name=f"ex{hp}_{e}_{s}")
                   for s in range(SC)] for e in range(2)]
            for s in range(SC):
                pss = [psQK.tile([128, R], F32, tag="psQK",
                                 name=f"psqk{hp}_{s}_{e}") for e in range(2)]
                for rc in range(2):
                    for e in range(2):
                        lo, hi = 64 * e, 64 * e + 64
                        nc.tensor.matmul(
                            pss[e][:, rc * 512:(rc + 1) * 512],
                            KT[hp][lo:hi, s * 128:(s + 1) * 128],
                            QT[hp][lo:hi, rc * 512:(rc + 1) * 512],
                            start=True, stop=True)
                for e in range(2):
                    nc.scalar.activation(ex[e][s][:], pss[e][:],
                                         AF.Exp, bias=mb_t[:, s:s + 1])
            return ex

        def attn_av(g, exs):
            """AV + denominators for group g (head pairs 2g, 2g+1).
            Per (hp, rc): two M=64 chains (heads -> col groups 0/64,
            concurrent) into ONE bank, evicted in a single [128,512]
            copy; same for the ones-chains -> packed sums tile."""
            sums = sumsp.tile([128, 2048], F32, tag="sums", name=f"sums{g}")
            for hpi in range(2):
                hp = 2 * g + hpi
                for rc in range(2):
                    rr = slice(rc * 512, rc * 512 + 512)
                    psO = psAV.tile([128, 512], F32, tag="psAV",
                                    name=f"psO{hp}_{rc}")
                    psS = psAV.tile([128, 512], F32, tag="psAV",
                                    name=f"psS{hp}_{rc}")
                    for s in range(SC):
                        for e in range(2):
                            rhs = exs[hp][e][s][:, rr]
                            nc.tensor.matmul(
                                psO[64 * e:64 * e + 64, :],
                                V2[s][:, (2 * hp + e) * 64:(2 * hp + e + 1) * 64],
                                rhs, start=(s == 0), stop=(s == SC - 1),
                                skip_group_check=True)
                    sec = hpi * 2 + rc
                    # O-evict emitted BEFORE the S-chains: the copy runs
                    # on DVE while the PE runs the S-chains, so the O
                    # bank is free when the next (hp, rc) starts.
                    nc.vector.tensor_copy(OT[hp][:, rr], psO[:])
                    for s in range(SC):
                        for e in range(2):
                            rhs = exs[hp][e][s][:, rr]
                            nc.tensor.matmul(
                                psS[64 * e:64 * e + 64, :],
                                ones_t[:, 0:64],
                                rhs, start=(s == 0), stop=(s == SC - 1),
                                skip_group_check=True)
                    nc.vector.tensor_copy(sums[:, sec * 512:(sec + 1) * 512],
                                          psS[:])
            return sums

        def normalize(g, sums):
            rcp = rcpp.tile([128, 2048], F32, tag="rcp", name=f"rcp{g}")
            nc.vector.reciprocal_approx_fast(rcp[:], sums[:])
            for hpi in range(2):
                hp = 2 * g + hpi
                for rc in range(2):
                    rr = slice(rc * 512, rc * 512 + 512)
                    sec = hpi * 2 + rc
                    nc.vector.tensor_mul(OT[hp][:, rr], OT[hp][:, rr],
                                         rcp[:, sec * 512:(sec + 1) * 512])

        # ================= schedule =================
        # Minimal critical path to the first Exp: kproj[0] -> qproj[0]
        # -> qk0.  The ACT Exp stream is the pipeline clock; kproj
        # chains and projections are spread across it as PE filler.
        # qk(n) is emitted as early as the exp pool allows (24 bufs =
        # 3 head pairs: qk(n) reuses qk(n-3)'s slots, whose readers are
        # av((n-3)//2)), so ACT never waits on PE priority inversions.
        ex = {}
        k_proj([0])
        q_proj([0])
        ex[0] = attn_qk(0)
        k_proj([1])
        q_proj([1])
        ex[1] = attn_qk(1)
        v_proj(0)
        k_proj([2, 3])
        q_proj([2, 3])
        ex[2] = attn_qk(2)
        s0 = attn_av(0, ex)
        normalize(0, s0)
        ex[3] = attn_qk(3)
        k_proj([4, 5])
        q_proj([4, 5])
        ex[4] = attn_qk(4)
        s1 = attn_av(1, ex)
        normalize(1, s1)
        ex[5] = attn_qk(5)
        k_proj([6, 7])
        p_wk.__exit__(None, None, None)
        q_proj([6, 7])
        ex[6] = attn_qk(6)
        v_proj(1)
        p_wv.__exit__(None, None, None)
        p_ctx.__exit__(None, None, None)
        p_wq.__exit__(None, None, None)
        p_xT.__exit__(None, None, None)
        # wp/bp loads late, into the region freed by ctx/wk/wv
        bp_t = constp.tile([128, D], F32, tag="bp")
        for k in range(KC):
            eng = nc.sync if k % 2 == 0 else nc.gpsimd
            eng.dma_start(wp_t[k][:], wp[k * 128:(k + 1) * 128, :])
        nc.gpsimd.dma_start(bp_t[:], bp_r[:])
        s2 = attn_av(2, ex)
        normalize(2, s2)
        ex[7] = attn_qk(7)
        s3 = attn_av(3, ex)
        normalize(3, s3)

        p_psAV.__exit__(None, None, None)
        p_psQK.__exit__(None, None, None)

        # ================= output projection =================
        p_psD = tc.tile_pool(name="psD", bufs=6, space="PSUM")
        psD = p_psD.__enter__()
        for rp in range(KC):
            for n in range(2):
                ps = psD.tile([128, 512], F32, tag="psD")
                for k in range(KC):
                    nc.tensor.matmul(
                        ps[:], OT[k][:, rp * 128:(rp + 1) * 128],
                        wp_t[k][:, n * 512:(n + 1) * 512],
                        start=(k == 0), stop=(k == KC - 1))
                yt = yp.tile([128, 512], F32, tag="y")
                nc.vector.tensor_add(yt[:], ps[:],
                                     bp_t[:, n * 512:(n + 1) * 512])
                eng = nc.sync if (rp * 2 + n) % 2 == 0 else nc.gpsimd
                eng.dma_start(
                    y[rp * 128:(rp + 1) * 128, n * 512:(n + 1) * 512], yt[:])
        p_psD.__exit__(None, None, None)
        p_psAB.__exit__(None, None, None)
        p_y.__exit__(None, None, None)
        p_rcp.__exit__(None, None, None)
        p_sums.__exit__(None, None, None)
        p_exp.__exit__(None, None, None)
        p_wp.__exit__(None, None, None)
        p_ot.__exit__(None, None, None)
        p_qt.__exit__(None, None, None)
        p_kv.__exit__(None, None, None)
        p_const.__exit__(None, None, None)

    nc.compile()
    return nc


def _get_nc():
    if "nc" not in _CACHE:
        _CACHE["nc"] = _build()
    return _CACHE["nc"]


def kernel(x, context, context_mask, Wq, bq, Wkv, bkv, Wp, bp):
    global last_results
    x = np.asarray(x, dtype=np.float32)
    context = np.asarray(context, dtype=np.float32)
    context_mask = np.asarray(context_mask)
    Wq = np.asarray(Wq, dtype=np.float32)
    bq = np.asarray(bq, dtype=np.float32)
    Wkv = np.asarray(Wkv, dtype=np.float32)
    bkv = np.asarray(bkv, dtype=np.float32)
    Wp = np.asarray(Wp, dtype=np.float32)
    bp = np.asarray(bp, dtype=np.float32)

    sc = 1.0 / np.sqrt(HD)

    def _m_major(w):
        # tile m = all k-chunks of output block m:
        # out[m*128+p, k*128+j] = w[k*128+p, m*128+j]
        return np.ascontiguousarray(
            w.reshape(KC, 128, KC, 128).transpose(2, 1, 0, 3).reshape(D, D))

    # kv reshape in the reference is [S, 2, H, Hd]: k cols = Wkv[:, :D]
    wq_h = _m_major((Wq * sc).astype(np.float16))
    bq_h = np.ascontiguousarray((bq * sc).reshape(KC, 128).T)
    wk_h = _m_major(Wkv[:, :D].astype(np.float16))
    bk_h = np.ascontiguousarray(bkv[:D].reshape(KC, 128).T)
    wv_h = np.ascontiguousarray((Wkv[:, D:] * 2.0**-10).astype(np.float16))
    bv = bkv[D:]
    wp_h = np.ascontiguousarray(Wp.astype(np.float16))
    bp_eff = bp + bv @ Wp          # softmax rows sum to 1
    bp_r = np.ascontiguousarray(
        np.broadcast_to(bp_eff.astype(np.float32), (128, D)))
    ones_h = np.full((128, 64), 2.0**-10, dtype=np.float16)

    in_maps = []
    for c in range(NCORE):
        b = c // 2
        r0 = (c % 2) * R
        in_maps.append({
            "xT": np.ascontiguousarray(x[b, r0:r0 + R, :].T.astype(np.float16)),
            "ctxT": np.ascontiguousarray(context[b].T.astype(np.float16)),
            "maskb": np.ascontiguousarray(
                np.where(context_mask[b], 0.0, NEG).astype(np.float32)
                .reshape(SC, 128).T),
            "wq": wq_h, "bq": bq_h,
            "wk": wk_h, "bk": bk_h,
            "wv": wv_h,
            "wp": wp_h, "bp_r": bp_r, "ones64": ones_h,
        })

    nc = _get_nc()
    res = run_bass_kernel_spmd(nc, in_maps, list(range(NCORE)),
                               trace=bool(os.environ.get("BASS_TRACE")))
    last_results = res

    out = np.empty((B, T, D), dtype=np.float32)
    for c in range(NCORE):
        b = c // 2
        r0 = (c % 2) * R
        out[b, r0:r0 + R, :] = res.results[c]["y"]
    return out


# revision 20
# speedup vs baseline: 1.0337x; 1.0337x over previous
"""Cross-attention kernel for TRN2, 8-core SPMD.

Reference op (B=4, T=2048, S=512, D=1024, H=16, Hd=64):
    q = (x @ Wq + bq); k,v = context @ Wkv + bkv
    out = softmax(q k^T / sqrt(Hd) + mask) @ v @ Wp + bp

Sharding: pure data-parallel over (batch, T/2): core c owns batch c//2,
query rows (c%2)*1024..+1024.  Each core recomputes K/V for its batch
(2x duplicated KV-proj work, zero collectives).  Weights replicated.

Device design (per core, R=1024 query rows), activations flow in
"transposed" space (feature on partitions, rows on free):
  - K proj -> KT [D,S], Q proj -> QT [D,R] (fp16).
  - V proj -> V2 [S, D] fp16 in natural head order, wv pre-scaled 2^-10.
  - scores^T [S,R] per head: K=64 fp16 matmuls, two heads of a pair in
    PE row-groups 0/64 (concurrent), one ACT Exp per [128,1024] psum
    tile with the context mask folded in as a per-partition bias.
  - AV per (head-pair, rc-half): two M=64 accumulation chains (heads in
    col-groups 0/64, concurrent) into ONE psum bank -> a single
    [128,512] DVE eviction lands both heads aligned with OT; a second
    bank takes the softmax denominators via a shared ones(64) lhsT.
  - Normalization: denominators packed [128,2048] per 2-pair group,
    reciprocal on DVE (reciprocal_approx_fast, ~51 ULP) -- the ACT
    engine runs ONLY Exp (one table load, no LUT thrash), then one
    in-place DVE multiply per (hp, rc).
  - Y [R,D] = OT^T @ Wp + bp, fp32 out, DMA'd out on both queues.

Schedule: the ACT Exp stream (64 x 1.15us) is the pipeline clock for
the back half; PE filler (Q proj, V proj, AV) is spread across it via
emission order so the PE never idles long enough for HAM to re-throttle.
Input DMA is ordered ctx+wk (split across both queues) -> xT+wq -> wv,
with wp loaded late into the freed region, so K proj streams as early
as possible.

Numerics: fp16 operands everywhere, fp32 PSUM accumulation.  Max-abs
error vs the fp32 reference ~1e-3 of max|out|.
"""
import os
import sys
import types

import numpy as np

import concourse.tile as tile
from concourse import bacc, mybir
from concourse.bass_utils import run_bass_kernel_spmd

F32 = mybir.dt.float32
F32R = mybir.dt.float32r
F16 = mybir.dt.float16
AF = mybir.ActivationFunctionType

B, T, S, D = 4, 2048, 512, 1024
H, HD = 16, 64
NCORE = 8
R = B * T // NCORE          # 1024 query rows per core
KC = D // 128               # 8 contraction chunks
SC = S // 128               # 4 context chunks
NP = H // 2                 # 8 head pairs
NEG = -60.0                 # mask bias (exp(-60) ~ 0)

_CACHE = {}
last_results = None         # BassKernelResults of the most recent run


def _install_ntff_hook():
    """antenv.axon_hooks is absent in this image; recreate it from the
    boot helper so BASS_TRACE=1 profiling works. Best-effort."""
    try:
        import antenv.axon_hooks  # noqa: F401
        return
    except ImportError:
        pass
    try:
        from trn_agent_boot.trn_boot import _ntff_profile_via_ctypes
        hook = _ntff_profile_via_ctypes("/opt/axon/libaxon_pjrt.so")
        mod = types.ModuleType("antenv.axon_hooks")
        mod.get_axon_ntff_profile_hook = lambda: hook
        sys.modules["antenv.axon_hooks"] = mod
    except Exception:
        pass


_install_ntff_hook()


def _build():
    nc = bacc.Bacc("TRN2", target_bir_lowering=False, debug=False,
                   num_devices=NCORE)

    xT = nc.dram_tensor("xT", [D, R], F16, kind="ExternalInput").ap()
    ctxT = nc.dram_tensor("ctxT", [D, S], F16, kind="ExternalInput").ap()
    maskb = nc.dram_tensor("maskb", [128, SC], F32, kind="ExternalInput").ap()
    wq = nc.dram_tensor("wq", [D, D], F16, kind="ExternalInput").ap()
    bq = nc.dram_tensor("bq", [128, KC], F32, kind="ExternalInput").ap()
    wk = nc.dram_tensor("wk", [D, D], F16, kind="ExternalInput").ap()
    bk = nc.dram_tensor("bk", [128, KC], F32, kind="ExternalInput").ap()
    wv = nc.dram_tensor("wv", [D, D], F16, kind="ExternalInput").ap()
    wp = nc.dram_tensor("wp", [D, D], F16, kind="ExternalInput").ap()
    bp_r = nc.dram_tensor("bp_r", [128, D], F32, kind="ExternalInput").ap()
    ones64 = nc.dram_tensor("ones64", [128, 64], F16, kind="ExternalInput").ap()
    y = nc.dram_tensor("y", [R, D], F32, kind="ExternalOutput").ap()

    with tile.TileContext(nc) as tc:
        # Pools close LIFO (stack bottom -> top):
        #   const < kv < qt < ot < wpp < exp < sums < rcp < y
        #   < psAB < psQK < psAV < xT < wq < ctx < wv < wk
        # closes: wk (post kproj), wv+ctx (post vproj1), wq+xT (post
        # qproj67); then psAV+psQK close after attention and psD opens
        # for the output projection.
        p_const = tc.tile_pool(name="const", bufs=1)
        p_kv = tc.tile_pool(name="kv", bufs=1)
        p_qt = tc.tile_pool(name="qt", bufs=1)
        p_ot = tc.tile_pool(name="ot", bufs=1)
        p_wp = tc.tile_pool(name="wpp", bufs=1)
        p_exp = tc.tile_pool(name="exp", bufs=22)
        p_sums = tc.tile_pool(name="sums", bufs=1)
        p_rcp = tc.tile_pool(name="rcp", bufs=1)
        p_y = tc.tile_pool(name="y", bufs=2)
        p_psAB = tc.tile_pool(name="psAB", bufs=2, space="PSUM")
        p_psQK = tc.tile_pool(name="psQK", bufs=2, space="PSUM")
        p_psAV = tc.tile_pool(name="psAV", bufs=2, space="PSUM")
        p_xT = tc.tile_pool(name="xTp", bufs=1)
        p_wq = tc.tile_pool(name="wqp", bufs=1)
        p_ctx = tc.tile_pool(name="ctxp", bufs=1)
        p_wv = tc.tile_pool(name="wvp", bufs=1)
        p_wk = tc.tile_pool(name="wkp", bufs=1)
        constp = p_const.__enter__()
        kvp = p_kv.__enter__()
        qtp = p_qt.__enter__()
        otp = p_ot.__enter__()
        wpp = p_wp.__enter__()
        expp = p_exp.__enter__()
        sumsp = p_sums.__enter__()
        rcpp = p_rcp.__enter__()
        yp = p_y.__enter__()
        psAB = p_psAB.__enter__()
        psQK = p_psQK.__enter__()
        psAV = p_psAV.__enter__()
        xTp = p_xT.__enter__()
        wqp = p_wq.__enter__()
        ctxp = p_ctx.__enter__()
        wvp = p_wv.__enter__()
        wkp = p_wk.__enter__()

        # ---- PE warm-up on a memset tile: HAM warm before loads land;
        # a dummy Exp right away pulls the ACT table load off the
        # critical path. ----
        warm_sb = constp.tile([128, 256], F32R, tag="warm_sb")
        nc.vector.memset(warm_sb[:].bitcast(F32), 0.0)
        dummy_e = constp.tile([128, 16], F16, tag="dummy_e")
        nc.scalar.activation(dummy_e[:], warm_sb[:].bitcast(F32)[:, 0:16],
                             AF.Exp)
        warm_ps = psAB.tile([128, 512], F32, tag="psAB")
        for w in range(36):
            nc.tensor.matmul(warm_ps[:, 0:256], warm_sb[:, 0:128], warm_sb[:],
                             start=True, stop=True, skip_group_check=True)

        # ---- small constants FIRST: the KT/QT/AV evictions depend on
        # them, and behind megabytes of weight DMA they stall the DVE
        # queue for tens of us. ----
        mb_t = constp.tile([128, SC], F32, tag="mb")
        nc.sync.dma_start(mb_t[:], maskb[:])
        bq_t = constp.tile([128, KC], F32, tag="bq")
        nc.sync.dma_start(bq_t[:], bq[:])
        bk_t = constp.tile([128, KC], F32, tag="bk")
        nc.gpsimd.dma_start(bk_t[:], bk[:])
        ones_t = constp.tile([128, 64], F16, tag="ones")
        nc.gpsimd.dma_start(ones_t[:], ones64[:])

        # ---- phase A loads.  wk/wq are m-major on the host (tile m =
        # all k-chunks of output block m), so chain m only waits for its
        # own 256KB tile.  xT loads in rc-halves so qproj's rc0 chain
        # can start after 1MB.  Order: ctx -> wk0 -> xT(rc0) -> wq0 ->
        # xT(rc1) -> wk1/wq1 -> rest, split across both DMA queues. ----
        ctx_t = [ctxp.tile([128, S], F16, tag=f"ctx{k}", name=f"ctx{k}")
                 for k in range(KC)]
        wk_t = [wkp.tile([128, D], F16, tag=f"wk{m}", name=f"wk{m}")
                for m in range(KC)]
        xT_t = [xTp.tile([128, R], F16, tag=f"xT{k}", name=f"xTs{k}")
                for k in range(KC)]
        wq_t = [wqp.tile([128, D], F16, tag=f"wq{m}", name=f"wqs{m}")
                for m in range(KC)]
        for k in range(KC):
            eng = nc.sync if k % 2 == 0 else nc.gpsimd
            eng.dma_start(ctx_t[k][:], ctxT[k * 128:(k + 1) * 128, :])
        nc.sync.dma_start(wk_t[0][:], wk[0:128, :])
        for k in range(KC):
            eng = nc.gpsimd if k % 2 == 0 else nc.sync
            eng.dma_start(xT_t[k][:, 0:512], xT[k * 128:(k + 1) * 128, 0:512])
        nc.gpsimd.dma_start(wq_t[0][:], wq[0:128, :])
        for k in range(KC):
            eng = nc.gpsimd if k % 2 == 0 else nc.sync
            eng.dma_start(xT_t[k][:, 512:1024],
                          xT[k * 128:(k + 1) * 128, 512:1024])
        nc.sync.dma_start(wk_t[1][:], wk[128:256, :])
        nc.gpsimd.dma_start(wq_t[1][:], wq[128:256, :])
        # wv next: vproj0's deadline (av0 at exp-hp1-end) is EARLIER
        # than kproj/qproj chains 2-7
        wv_t = [wvp.tile([128, D], F16, tag=f"wv{k}", name=f"wv{k}")
                for k in range(KC)]
        for k in range(KC):
            eng = nc.sync if k % 2 == 0 else nc.gpsimd
            eng.dma_start(wv_t[k][:], wv[k * 128:(k + 1) * 128, :])
        for m in range(2, KC):
            eng = nc.sync if m % 2 == 0 else nc.gpsimd
            eng.dma_start(wk_t[m][:], wk[m * 128:(m + 1) * 128, :])
            eng2 = nc.gpsimd if m % 2 == 0 else nc.sync
            eng2.dma_start(wq_t[m][:], wq[m * 128:(m + 1) * 128, :])

        # ---- persistent attention operands (fp16) ----
        KT = [kvp.tile([128, S], F16, tag=f"KT{m}", name=f"KT{m}")
              for m in range(KC)]
        V2 = [kvp.tile([128, H * HD], F16, tag=f"V2{s}", name=f"V2{s}")
              for s in range(SC)]
        QT = [qtp.tile([128, R], F16, tag=f"QT{m}", name=f"QT{m}")
              for m in range(KC)]
        OT = [otp.tile([128, R], F16, tag=f"OT{m}", name=f"OT{m}")
              for m in range(KC)]
        wp_t = [wpp.tile([128, D], F16, tag=f"wp{k}", name=f"wps{k}")
                for k in range(KC)]

        # ---- emitters.  wk_t/wq_t tile m holds ALL k-chunks of output
        # block m: lhsT for the k-th matmul is tile[m][:, k*128:+128].
        def k_proj(ms):
            for m in ms:
                ps = psAB.tile([128, S], F32, tag="psAB")
                for k in range(KC):
                    nc.tensor.matmul(ps[:], wk_t[m][:, k * 128:(k + 1) * 128],
                                     ctx_t[k][:],
                                     start=(k == 0), stop=(k == KC - 1))
                nc.vector.tensor_scalar_add(KT[m][:], ps[:], bk_t[:, m:m + 1])

        def q_proj(ms):
            for m in ms:
                for rc in range(2):
                    ps = psAB.tile([128, 512], F32, tag="psAB")
                    for k in range(KC):
                        nc.tensor.matmul(
                            ps[:], wq_t[m][:, k * 128:(k + 1) * 128],
                            xT_t[k][:, rc * 512:(rc + 1) * 512],
                            start=(k == 0), stop=(k == KC - 1))
                    nc.vector.tensor_scalar_add(
                        QT[m][:, rc * 512:(rc + 1) * 512], ps[:],
                        bq_t[:, m:m + 1])

        def v_proj(n):
            for s in range(SC):
                ps = psAB.tile([128, 512], F32, tag="psAB")
                for k in range(KC):
                    nc.tensor.matmul(ps[:], ctx_t[k][:, s * 128:(s + 1) * 128],
                                     wv_t[k][:, n * 512:(n + 1) * 512],
                                     start=(k == 0), stop=(k == KC - 1))
                nc.vector.tensor_copy(V2[s][:, n * 512:(n + 1) * 512], ps[:])

        def attn_qk(hp):
            """scores^T + exp for one head pair; both heads run in PE
            row-groups 0/64 concurrently.  Returns ex[e][s] fp16 tiles."""
            ex = [[expp.tile([128, R], F16, tag="exp", name=f"ex{hp}_{e}_{s}")
                   for s in range(SC)] for e in range(2)]
            for s in range(SC):
                pss = [psQK.tile([128, R], F32, tag="psQK",
                                 name=f"psqk{hp}_{s}_{e}") for e in range(2)]
                for rc in range(2):
                    for e in range(2):
                        lo, hi = 64 * e, 64 * e + 64
                        nc.tensor.matmul(
                            pss[e][:, rc * 512:(rc + 1) * 512],
                            KT[hp][lo:hi, s * 128:(s + 1) * 128],
                            QT[hp][lo:hi, rc * 512:(rc + 1) * 512],
                            start=True, stop=True)
                for e in range(2):
                    nc.scalar.activation(ex[e][s][:], pss[e][:],
                                         AF.Exp, bias=mb_t[:, s:s + 1])
            return ex

        def attn_av(g, exs):
            """AV + denominators for group g (head pairs 2g, 2g+1).
            Per (hp, rc): two M=64 chains (heads -> col groups 0/64,
            concurrent) into ONE bank, evicted in a single [128,512]
            copy; same for the ones-chains -> packed sums tile."""
            sums = sumsp.tile([128, 2048], F32, tag="sums", name=f"sums{g}")
            for hpi in range(2):
                hp = 2 * g + hpi
                for rc in range(2):
                    rr = slice(rc * 512, rc * 512 + 512)
                    psO = psAV.tile([128, 512], F32, tag="psAV",
                                    name=f"psO{hp}_{rc}")
                    psS = psAV.tile([128, 512], F32, tag="psAV",
                                    name=f"psS{hp}_{rc}")
                    for s in range(SC):
                        for e in range(2):
                            rhs = exs[hp][e][s][:, rr]
                            nc.tensor.matmul(
                                psO[64 * e:64 * e + 64, :],
                                V2[s][:, (2 * hp + e) * 64:(2 * hp + e + 1) * 64],
                                rhs, start=(s == 0), stop=(s == SC - 1),
                                skip_group_check=True)
                    sec = hpi * 2 + rc
                    # O-evict emitted BEFORE the S-chains: the copy runs
                    # on DVE while the PE runs the S-chains, so the O
                    # bank is free when the next (hp, rc) starts.
                    nc.vector.tensor_copy(OT[hp][:, rr], psO[:])
                    for s in range(SC):
                        for e in range(2):
                            rhs = exs[hp][e][s][:, rr]
                            nc.tensor.matmul(
                                psS[64 * e:64 * e + 64, :],
                                ones_t[:, 0:64],
                                rhs, start=(s == 0), stop=(s == SC - 1),
                                skip_group_check=True)
                    nc.vector.tensor_copy(sums[:, sec * 512:(sec + 1) * 512],
                                          psS[:])
            return sums

        def normalize(g, sums):
            rcp = rcpp.tile([128, 2048], F32, tag="rcp", name=f"rcp{g}")
            nc.vector.reciprocal_approx_fast(rcp[:], sums[:])
            for hpi in range(2):
                hp = 2 * g + hpi
                for rc in range(2):
                    rr = slice(rc * 512, rc * 512 + 512)
                    sec = hpi * 2 + rc
                    nc.vector.tensor_mul(OT[hp][:, rr], OT[hp][:, rr],
                                         rcp[:, sec * 512:(sec + 1) * 512])

        # ================= schedule =================
        # Minimal critical path to the first Exp: kproj[0] -> qproj[0]
        # -> qk0.  The ACT Exp stream is the pipeline clock; kproj
        # chains and projections are spread across it as PE filler.
        # qk(n) is emitted as early as the exp pool allows (24 bufs =
        # 3 head pairs: qk(n) reuses qk(n-3)'s slots, whose readers are
        # av((n-3)//2)), so ACT never waits on PE priority inversions.
        ex = {}
        k_proj([0])
        q_proj([0])
        ex[0] = attn_qk(0)
        fill_ps = psAV.tile([128, 512], F32, tag="psAV", name="fill_ps")
        for w in range(16):
            nc.tensor.matmul(fill_ps[:, 0:256], warm_sb[:, 0:128],
                             warm_sb[:], start=True, stop=True,
                             skip_group_check=True)
        k_proj([1])
        q_proj([1])
        ex[1] = attn_qk(1)
        v_proj(0)
        k_proj([2, 3])
        q_proj([2, 3])
        ex[2] = attn_qk(2)
        s0 = attn_av(0, ex)
        normalize(0, s0)
        ex[3] = attn_qk(3)
        k_proj([4, 5])
        q_proj([4, 5])
        ex[4] = attn_qk(4)
        s1 = attn_av(1, ex)
        normalize(1, s1)
        ex[5] = attn_qk(5)
        k_proj([6, 7])
        p_wk.__exit__(None, None, None)
        q_proj([6, 7])
        ex[6] = attn_qk(6)
        v_proj(1)
        p_wv.__exit__(None, None, None)
        p_ctx.__exit__(None, None, None)
        p_wq.__exit__(None, None, None)
        p_xT.__exit__(None, None, None)
        # wp/bp loads late, into the region freed by ctx/wk/wv
        bp_t = constp.tile([128, D], F32, tag="bp")
        for k in range(KC):
            eng = nc.sync if k % 2 == 0 else nc.gpsimd
            eng.dma_start(wp_t[k][:], wp[k * 128:(k + 1) * 128, :])
        nc.gpsimd.dma_start(bp_t[:], bp_r[:])
        s2 = attn_av(2, ex)
        normalize(2, s2)
        ex[7] = attn_qk(7)
        s3 = attn_av(3, ex)
        normalize(3, s3)

        p_psAV.__exit__(None, None, None)
        p_psQK.__exit__(None, None, None)

        # ================= output projection =================
        p_psD = tc.tile_pool(name="psD", bufs=6, space="PSUM")
        psD = p_psD.__enter__()
        for rp in range(KC):
            for n in range(2):
                ps = psD.tile([128, 512], F32, tag="psD")
                for k in range(KC):
                    nc.tensor.matmul(
                        ps[:], OT[k][:, rp * 128:(rp + 1) * 128],
                        wp_t[k][:, n * 512:(n + 1) * 512],
                        start=(k == 0), stop=(k == KC - 1))
                yt = yp.tile([128, 512], F32, tag="y")
                nc.vector.tensor_add(yt[:], ps[:],
                                     bp_t[:, n * 512:(n + 1) * 512])
                eng = nc.sync if (rp * 2 + n) % 2 == 0 else nc.gpsimd
                eng.dma_start(
                    y[rp * 128:(rp + 1) * 128, n * 512:(n + 1) * 512], yt[:])
        p_psD.__exit__(None, None, None)
        p_psAB.__exit__(None, None, None)
        p_y.__exit__(None, None, None)
        p_rcp.__exit__(None, None, None)
        p_sums.__exit__(None, None, None)
        p_exp.__exit__(None, None, None)
        p_wp.__exit__(None, None, None)
        p_ot.__exit__(None, None, None)
        p_qt.__exit__(None, None, None)
        p_kv.__exit__(None, None, None)
        p_const.__exit__(None, None, None)

    nc.compile()
    return nc


def _get_nc():
    if "nc" not in _CACHE:
        _CACHE["nc"] = _build()
    return _CACHE["nc"]


def kernel(x, context, context_mask, Wq, bq, Wkv, bkv, Wp, bp):
    global last_results
    x = np.asarray(x, dtype=np.float32)
    context = np.asarray(context, dtype=np.float32)
    context_mask = np.asarray(context_mask)
    Wq = np.asarray(Wq, dtype=np.float32)
    bq = np.asarray(bq, dtype=np.float32)
    Wkv = np.asarray(Wkv, dtype=np.float32)
    bkv = np.asarray(bkv, dtype=np.float32)
    Wp = np.asarray(Wp, dtype=np.float32)
    bp = np.asarray(bp, dtype=np.float32)

    sc = 1.0 / np.sqrt(HD)

    def _m_major(w):
        # tile m = all k-chunks of output block m:
        # out[m*128+p, k*128+j] = w[k*128+p, m*128+j]
        return np.ascontiguousarray(
            w.reshape(KC, 128, KC, 128).transpose(2, 1, 0, 3).reshape(D, D))

    # kv reshape in the reference is [S, 2, H, Hd]: k cols = Wkv[:, :D]
    wq_h = _m_major((Wq * sc).astype(np.float16))
    bq_h = np.ascontiguousarray((bq * sc).reshape(KC, 128).T)
    wk_h = _m_major(Wkv[:, :D].astype(np.float16))
    bk_h = np.ascontiguousarray(bkv[:D].reshape(KC, 128).T)
    wv_h = np.ascontiguousarray((Wkv[:, D:] * 2.0**-10).astype(np.float16))
    bv = bkv[D:]
    wp_h = np.ascontiguousarray(Wp.astype(np.float16))
    bp_eff = bp + bv @ Wp          # softmax rows sum to 1
    bp_r = np.ascontiguousarray(
        np.broadcast_to(bp_eff.astype(np.float32), (128, D)))
    ones_h = np.full((128, 64), 2.0**-10, dtype=np.float16)

    in_maps = []
    for c in range(NCORE):
        b = c // 2
        r0 = (c % 2) * R
        in_maps.append({
            "xT": np.ascontiguousarray(x[b, r0:r0 + R, :].T.astype(np.float16)),
            "ctxT": np.ascontiguousarray(context[b].T.astype(np.float16)),
            "maskb": np.ascontiguousarray(
                np.where(context_mask[b], 0.0, NEG).astype(np.float32)
                .reshape(SC, 128).T),
            "wq": wq_h, "bq": bq_h,
            "wk": wk_h, "bk": bk_h,
            "wv": wv_h,
            "wp": wp_h, "bp_r": bp_r, "ones64": ones_h,
        })

    nc = _get_nc()
    res = run_bass_kernel_spmd(nc, in_maps, list(range(NCORE)),
                               trace=bool(os.environ.get("BASS_TRACE")))
    last_results = res

    out = np.empty((B, T, D), dtype=np.float32)
    for c in range(NCORE):
        b = c // 2
        r0 = (c % 2) * R
        out[b, r0:r0 + R, :] = res.results[c]["y"]
    return out
